# revision 15
# baseline (speedup 1.0000x reference)
"""Trainium2 Bass kernel for CoreferenceResolution.

Math: logits[b,p] = relu(concat(M[b,i], M[b,j], ED[e]) @ W1 + b1) @ W2 + b2
Decomposed as: relu(U[b,i] + V[b,j] + E'[e]) @ W2 + b2 with
  U = M @ W1[:768], V = M @ W1[768:1536], E' = ED @ W1[1536:] + b1.

All indexed lookups are done on the TensorEngine as one-hot matmuls in a
transposed layout (preH^T[h, pair] accumulated in PSUM), which also fuses
the three adds into PSUM accumulation and the relu into the PSUM drain:

  preH^T[hc] = sum_s table_s[ments, hc].T @ onehot_s[ments, pairs]

Static structure (8 cores = 2 batches x 4 V-buckets):
 - pairs are assigned to the core owning b's mention chunk-of-512 (the core's
   mentions are host-reordered so its V bucket is rows 0..511 -> V uses 4
   static chunk slots).
 - within a core, pairs are placed into per-a-chunk quota ranges so each
   512-pair tile needs only the 2-3 statically-known U chunks covering its
   quota window; overflow pairs go to one slop tile with all 16 U slots.
 - E' spans 3 static chunks.
One-hot masks are built host-side (pure index marshalling) and streamed as
bf16; dummy slots have all-zero mask columns.
"""

import math
import sys

sys.path.insert(0, "/opt/trn_rl_repo")

import numpy as np

HIDDEN = 768
HC = 6                        # hidden chunks of 128
B = 2
N_MENT = 2000
MENT_PAD = 2048
M_CHUNKS = 16
N_PAIRS = 40000
ED_COUNT = 300
ED_PAD = 384
E_CHUNKS = 3
META = 25
W1_ROWS_PAD = 1664            # 1561 -> 13 chunks of 128
W1_CHUNKS = 13
N_CORES = 8
SLICES = 4                    # V buckets (of 512 mentions) per batch
V_CHUNKS = 4                  # mention chunks per V bucket
T = 512                       # pairs per tile

N_EXP = 10240                 # expected pairs per core


def _quotas():
    """Per-a-chunk quota (same for every core; mean + 2.5 sigma slack)."""
    qs = []
    for c in range(M_CHUNKS):
        size = min(128, max(0, N_MENT - c * 128))
        p = size / N_MENT
        mean = N_EXP * p
        qs.append(int(math.ceil(mean + 2.5 * math.sqrt(mean))))
    return qs


QUOTAS = _quotas()
QCUM = [0]
for q in QUOTAS:
    QCUM.append(QCUM[-1] + q)
NT_Q = (QCUM[-1] + T - 1) // T        # quota tiles
NT_ALL = NT_Q + 1                     # + one slop tile (all 16 U chunks)
SLOP_CAP = T


def _tile_windows():
    """U-chunk window for each quota tile (static)."""
    wins = []
    for t in range(NT_Q):
        lo, hi = t * T, (t + 1) * T
        w = [c for c in range(M_CHUNKS) if QCUM[c] < hi and QCUM[c + 1] > lo]
        wins.append(w)
    wins.append(list(range(M_CHUNKS)))  # slop tile
    return wins


WINDOWS = _tile_windows()

# flat static slot list: (tile, kind, chunk); kind: 0=U, 1=V, 2=E
SLOTS = []
SLOT_BASE = []                # first slot index of each tile
for t in range(NT_ALL):
    SLOT_BASE.append(len(SLOTS))
    for c in WINDOWS[t]:
        SLOTS.append((t, 0, c))
    for j in range(V_CHUNKS):
        SLOTS.append((t, 1, j))
    for j in range(E_CHUNKS):
        SLOTS.append((t, 2, j))
SLOT_BASE.append(len(SLOTS))
N_SLOTS = len(SLOTS)

_COMPILED = None


def _build():
    import concourse.mybir as mybir
    import concourse.tile as tile
    from concourse import bacc
    from concourse.bass import ts

    dt = mybir.dt
    nc = bacc.Bacc("TRN2", target_bir_lowering=False, debug=False,
                   num_devices=N_CORES)

    ments_d = nc.dram_tensor("ments", [MENT_PAD, HIDDEN], dt.float32,
                             kind="ExternalInput").ap()
    w1_d = nc.dram_tensor("w1p", [W1_ROWS_PAD, HIDDEN], dt.float32,
                          kind="ExternalInput").ap()
    w2_d = nc.dram_tensor("w2", [HIDDEN], dt.float32,
                          kind="ExternalInput").ap()
    b1_d = nc.dram_tensor("b1", [HIDDEN], dt.float32,
                          kind="ExternalInput").ap()
    b2_d = nc.dram_tensor("b2", [1], dt.float32, kind="ExternalInput").ap()
    edt_d = nc.dram_tensor("edt", [32, ED_PAD], dt.float32,
                           kind="ExternalInput").ap()
    oh_d = nc.dram_tensor("oh", [128, N_SLOTS, T], dt.bfloat16,
                          kind="ExternalInput").ap()
    out_d = nc.dram_tensor("out", [NT_ALL * T], dt.float32,
                           kind="ExternalOutput").ap()
    scratch_d = nc.dram_tensor("scratch", [MENT_PAD, HIDDEN], dt.bfloat16).ap()

    with tile.TileContext(nc) as tc:
        with (
            tc.tile_pool(name="const", bufs=1) as cpool,
            tc.tile_pool(name="tables", bufs=1) as tpool,
        ):
            w1_sb = cpool.tile([128, W1_CHUNKS * HIDDEN], dt.bfloat16)
            w2b = cpool.tile([128, HC], dt.bfloat16)
            b1b = cpool.tile([128, HIDDEN], dt.float32)
            b2_sb = cpool.tile([1, 1], dt.float32)
            edt_sb = cpool.tile([32, ED_PAD], dt.bfloat16)

            u_sb = tpool.tile([128, M_CHUNKS * HIDDEN], dt.bfloat16)
            v_sb = tpool.tile([128, V_CHUNKS * HIDDEN], dt.bfloat16)
            e_sb = tpool.tile([128, E_CHUNKS * HIDDEN], dt.bfloat16)

            with (
                tc.tile_pool(name="stage", bufs=4) as spool,
                tc.tile_pool(name="small", bufs=1) as smpool,
                tc.tile_pool(name="mentT", bufs=1) as mtpool,
                tc.tile_pool(name="psA", bufs=4, space="PSUM") as psA,
            ):
                # ---- loads + casts ----
                nc.sync.dma_start(b2_sb[:], b2_d[:])

                for c in range(W1_CHUNKS):
                    st = spool.tile([128, HIDDEN], dt.float32, tag="st")
                    nc.sync.dma_start(st[:], w1_d[ts(c, 128), :])
                    nc.scalar.copy(w1_sb[:, ts(c, HIDDEN)], st[:])

                ment_stage = mtpool.tile([128, M_CHUNKS * HIDDEN], dt.bfloat16)
                for r in range(M_CHUNKS):
                    st = spool.tile([128, HIDDEN], dt.float32, tag="st")
                    nc.sync.dma_start(st[:], ments_d[ts(r, 128), :])
                    nc.vector.tensor_copy(ment_stage[:, ts(r, HIDDEN)], st[:])

                # mentions^T via DRAM round-trip + xbar transpose DMA
                nc.sync.dma_start(
                    scratch_d.rearrange("(r p) h -> p r h", p=128),
                    ment_stage[:].rearrange("p (r h) -> p r h", h=HIDDEN))
                mentT = []
                for k in range(HC):
                    mt = mtpool.tile([128, MENT_PAD], dt.bfloat16,
                                     tag=f"mt{k}", name=f"mentT{k}")
                    nc.sync.dma_start(mt[:], scratch_d[:, ts(k, 128)],
                                      transpose=True)
                    mentT.append(mt)

                w2st = smpool.tile([128, HC], dt.float32)
                nc.sync.dma_start(w2st[:], w2_d.rearrange("(c p) -> p c", p=128))
                nc.vector.tensor_copy(w2b[:], w2st[:])

                b1st = smpool.tile([1, HIDDEN], dt.float32)
                nc.sync.dma_start(b1st[:], b1_d[None, :])
                nc.gpsimd.partition_broadcast(b1b[:], b1st[:])

                edst = smpool.tile([32, ED_PAD], dt.float32)
                nc.sync.dma_start(edst[:], edt_d[:])
                nc.vector.tensor_copy(edt_sb[:], edst[:])

                # ---- E' = ed^T.T @ W1c + b1 ----
                w1c_off = 12 * HIDDEN
                for m in range(E_CHUNKS):
                    p5 = psA.tile([128, 512], dt.float32, tag="p5")
                    p2 = psA.tile([128, 256], dt.float32, tag="p2")
                    lhs = edt_sb[:META, ts(m, 128)]
                    nc.tensor.matmul(p5[:], lhs,
                                     w1_sb[:META, w1c_off:w1c_off + 512],
                                     start=True, stop=True)
                    nc.tensor.matmul(p2[:], lhs,
                                     w1_sb[:META, w1c_off + 512:w1c_off + HIDDEN],
                                     start=True, stop=True)
                    nc.vector.tensor_add(e_sb[:, m * HIDDEN:m * HIDDEN + 512],
                                         p5[:], b1b[:, :512])
                    nc.vector.tensor_add(
                        e_sb[:, m * HIDDEN + 512:(m + 1) * HIDDEN],
                        p2[:], b1b[:, 512:])

                # ---- U (16 chunks) and V (first 4 chunks) projections ----
                for r in range(M_CHUNKS):
                    u5 = psA.tile([128, 512], dt.float32, tag="p5")
                    u2 = psA.tile([128, 256], dt.float32, tag="p2")
                    do_v = r < V_CHUNKS
                    if do_v:
                        v5 = psA.tile([128, 512], dt.float32, tag="p5")
                        v2 = psA.tile([128, 256], dt.float32, tag="p2")
                    for k in range(HC):
                        lhs = mentT[k][:, ts(r, 128)]
                        st0, sp1 = (k == 0), (k == HC - 1)
                        ua = k * HIDDEN
                        va = (HC + k) * HIDDEN
                        nc.tensor.matmul(u5[:], lhs, w1_sb[:, ua:ua + 512],
                                         start=st0, stop=sp1)
                        nc.tensor.matmul(u2[:], lhs,
                                         w1_sb[:, ua + 512:ua + HIDDEN],
                                         start=st0, stop=sp1)
                        if do_v:
                            nc.tensor.matmul(v5[:], lhs, w1_sb[:, va:va + 512],
                                             start=st0, stop=sp1)
                            nc.tensor.matmul(v2[:], lhs,
                                             w1_sb[:, va + 512:va + HIDDEN],
                                             start=st0, stop=sp1)
                    ro = r * HIDDEN
                    nc.vector.tensor_copy(u_sb[:, ro:ro + 512], u5[:])
                    nc.vector.tensor_copy(u_sb[:, ro + 512:ro + HIDDEN], u2[:])
                    if do_v:
                        nc.scalar.copy(v_sb[:, ro:ro + 512], v5[:])
                        nc.scalar.copy(v_sb[:, ro + 512:ro + HIDDEN], v2[:])

            # ---- pair tiles: one-hot expansion + relu + dot ----
            with (
                tc.tile_pool(name="oh", bufs=2) as ohpool,
                tc.tile_pool(name="h", bufs=4) as hpool,
                tc.tile_pool(name="o", bufs=2) as opool,
                tc.tile_pool(name="psD", bufs=4, space="PSUM") as psD,
                tc.tile_pool(name="psL", bufs=2, space="PSUM") as psL,
            ):
                relu = mybir.ActivationFunctionType.Relu
                ident = mybir.ActivationFunctionType.Identity
                for t in range(NT_ALL):
                    ns = SLOT_BASE[t + 1] - SLOT_BASE[t]
                    oh_t = ohpool.tile([128, ns, T], dt.bfloat16, tag="oh")
                    nc.sync.dma_start(
                        oh_t[:], oh_d[:, SLOT_BASE[t]:SLOT_BASE[t + 1], :])
                    pl = psL.tile([1, T], dt.float32, tag="pl")
                    for hc in range(HC):
                        ph = psD.tile([128, T], dt.float32, tag="ph")
                        for s in range(ns):
                            _, kind, c = SLOTS[SLOT_BASE[t] + s]
                            tab = (u_sb, v_sb, e_sb)[kind]
                            lhs = tab[:, c * HIDDEN + hc * 128:
                                      c * HIDDEN + (hc + 1) * 128]
                            nc.tensor.matmul(ph[:], lhs, oh_t[:, s, :],
                                             start=(s == 0), stop=(s == ns - 1))
                        h_sb = hpool.tile([128, T], dt.bfloat16, tag="h")
                        nc.scalar.activation(h_sb[:], ph[:], relu)
                        nc.tensor.matmul(pl[:], w2b[:, hc:hc + 1], h_sb[:],
                                         start=(hc == 0), stop=(hc == HC - 1))
                    lt = opool.tile([1, T], dt.float32, tag="lt")
                    nc.scalar.activation(lt[:], pl[:], ident,
                                         bias=b2_sb[:1, :1])
                    nc.sync.dma_start(out_d[ts(t, T)], lt[:])

    nc.compile()
    return nc


def _get_compiled():
    global _COMPILED
    if _COMPILED is None:
        _COMPILED = _build()
    return _COMPILED


def _assign(core_pairs_a):
    """Place pairs (by a-chunk) into quota slots; return (pos[i], ok) lists.

    core_pairs_a: int array of a' indices. Returns positions array into
    [0, NT_ALL*T) with -1 never (asserts capacity).
    """
    n = len(core_pairs_a)
    pos = np.full(n, -1, np.int64)
    ah = core_pairs_a // 128
    slop_next = NT_Q * T
    for c in range(M_CHUNKS):
        idx = np.nonzero(ah == c)[0]
        k = min(len(idx), QUOTAS[c])
        pos[idx[:k]] = QCUM[c] + np.arange(k)
        for i in idx[k:]:
            assert slop_next < NT_Q * T + SLOP_CAP, "slop overflow"
            pos[i] = slop_next
            slop_next += 1
    return pos


def make_in_maps(mention_reprs, coref_mention_pairs, coref_eds, ed_table,
                 W1, b1, W2, b2):
    import ml_dtypes

    mention_reprs = np.asarray(mention_reprs, dtype=np.float32)
    pairs = np.asarray(coref_mention_pairs).astype(np.int64)
    eds = np.asarray(coref_eds).astype(np.int64)
    W1 = np.asarray(W1, dtype=np.float32)
    W2 = np.asarray(W2, dtype=np.float32)
    b1 = np.asarray(b1, dtype=np.float32)
    b2 = np.asarray(b2, dtype=np.float32)
    ed_table = np.asarray(ed_table, dtype=np.float32)

    w1p = np.zeros((W1_ROWS_PAD, HIDDEN), np.float32)
    w1p[:W1.shape[0]] = W1
    edt = np.zeros((32, ED_PAD), np.float32)
    edt[:META, :ed_table.shape[0]] = ed_table.T

    shared = {
        "w1p": w1p,
        "w2": W2.reshape(HIDDEN),
        "b1": b1.reshape(HIDDEN),
        "b2": b2.reshape(1),
        "edt": edt,
    }

    in_maps = []
    placements = []               # (orig_positions, device_positions) per core
    for core in range(N_CORES):
        b = core // SLICES
        q = core % SLICES
        # mention reorder: V bucket rows first
        bucket = np.arange(512 * q, min(512 * (q + 1), N_MENT))
        rest = np.concatenate([np.arange(0, 512 * q),
                               np.arange(min(512 * (q + 1), N_MENT), N_MENT)])
        perm = np.concatenate([bucket, rest])        # new_row -> old_row
        inv_perm = np.empty(N_MENT, np.int64)
        inv_perm[perm] = np.arange(N_MENT)

        ments = np.zeros((MENT_PAD, HIDDEN), np.float32)
        ments[:N_MENT] = mention_reprs[b][perm]

        bsel = (pairs[b, :, 1] >= 512 * q) & (pairs[b, :, 1] < 512 * (q + 1))
        psel = np.nonzero(bsel)[0]
        a_new = inv_perm[pairs[b, psel, 0]]
        b_loc = inv_perm[pairs[b, psel, 1]]          # in [0, 512)
        e_val = eds[b, psel]

        pos = _assign(a_new)

        # one-hot mask tensor [128, N_SLOTS, T]
        oh = np.zeros((128, N_SLOTS, T), ml_dtypes.bfloat16)
        tile_i = pos // T
        col_i = pos % T
        # map (tile, kind, chunk) -> slot id
        slot_of = {}
        for s, (t, kind, c) in enumerate(SLOTS):
            slot_of[(t, kind, c)] = s
        ah = a_new // 128
        al = a_new % 128
        bh = b_loc // 128
        bl = b_loc % 128
        eh = e_val // 128
        el = e_val % 128
        for i in range(len(psel)):
            t = tile_i[i]
            su = slot_of.get((t, 0, int(ah[i])))
            assert su is not None, (t, ah[i])
            oh[al[i], su, col_i[i]] = 1
            oh[bl[i], slot_of[(t, 1, int(bh[i]))], col_i[i]] = 1
            oh[el[i], slot_of[(t, 2, int(eh[i]))], col_i[i]] = 1

        placements.append((psel, b, pos))
        in_maps.append({"ments": ments, "oh": oh, **shared})
    make_in_maps.placements = placements
    return in_maps


def unshard(results, placements):
    out = np.zeros((B, N_PAIRS), np.float32)
    for core in range(N_CORES):
        psel, b, pos = placements[core]
        vals = results[core]["out"]
        out[b, psel] = vals[pos]
    return out


def kernel(**inputs):
    from concourse.bass_utils import run_bass_kernel_spmd

    nc = _get_compiled()
    in_maps = make_in_maps(**inputs)
    placements = make_in_maps.placements
    res = run_bass_kernel_spmd(nc, in_maps, list(range(N_CORES)))
    return unshard(res.results, placements)


# revision 22
# speedup vs baseline: 3.0211x; 3.0211x over previous
"""Trainium2 Bass kernel for CoreferenceResolution.

Math: logits[b,p] = relu(concat(M[b,i], M[b,j], ED[e]) @ W1 + b1) @ W2 + b2
Decomposed as: relu(U[b,i] + V[b,j] + E'[e]) @ W2 + b2 with
  U = M @ W1[:768], V = M @ W1[768:1536], E' = ED @ W1[1536:] + b1
  (b1 folded into E' by appending an all-ones row to ED^T and b1 to W1c).

All indexed lookups run on the TensorEngine as one-hot matmuls in a
transposed layout (preH^T[h, pair] accumulated in PSUM): the three adds fuse
into PSUM accumulation and relu fuses into the PSUM drain on ScalarE.
One-hot masks are built on-device per tile: PE broadcasts a per-column
lane-id row (K=1 matmul with a ones vector) into PSUM, then VectorE
is_equal against an iota per-partition scalar produces the bf16 mask.

Static structure (8 cores = 2 batches x 4 V-buckets):
 - pairs go to the core owning b's mention chunk-of-512; each core's mention
   table is host-reordered so its V bucket is rows 0..511 (V = 4 static
   chunk slots, and V is only projected for those 512 mentions).
 - within a core, pairs are placed into per-a-chunk quota ranges so each
   512-pair tile needs only the 1-2 statically-known U chunks covering its
   quota window; overflow goes to one slop tile with all 16 U slots.
 - E' spans 3 static chunks.
Host-side work is index marshalling only: per-slot lane values (bf16 codes
0..127, 255 = no match), mention reorder, and bf16 casts of the weights
(the kernel computes in bf16 regardless).
"""

import math
import sys

sys.path.insert(0, "/opt/trn_rl_repo")

import numpy as np

HIDDEN = 768
HC = 6                        # hidden chunks of 128
B = 2
N_MENT = 2000
MENT_PAD = 2048
M_CHUNKS = 16
N_PAIRS = 40000
ED_COUNT = 300
ED_PAD = 384
E_CHUNKS = 3
META = 25
W1_ROWS_PAD = 1664            # 1561 -> 13 chunks of 128
W1_CHUNKS = 13
N_CORES = 8
SLICES = 4                    # V buckets (of 512 mentions) per batch
V_CHUNKS = 4                  # mention chunks per V bucket
T = 512                       # pairs per tile

N_EXP = 10240                 # expected pairs per core
NOMATCH = 255.0               # lane code that matches no partition


def _quotas():
    """Per-a-chunk quota (same for every core; mean + 2.5 sigma slack)."""
    qs = []
    for c in range(M_CHUNKS):
        size = min(128, max(0, N_MENT - c * 128))
        p = size / N_MENT
        mean = N_EXP * p
        qs.append(int(math.ceil(mean + 2.5 * math.sqrt(mean))))
    return qs


QUOTAS = _quotas()
QCUM = [0]
for q in QUOTAS:
    QCUM.append(QCUM[-1] + q)
NT_Q = (QCUM[-1] + T - 1) // T        # quota tiles
NT_ALL = NT_Q + 1                     # + one slop tile (all 16 U chunks)
SLOP_CAP = T


def _tile_windows():
    wins = []
    for t in range(NT_Q):
        lo, hi = t * T, (t + 1) * T
        w = [c for c in range(M_CHUNKS) if QCUM[c] < hi and QCUM[c + 1] > lo]
        wins.append(w)
    wins.append(list(range(M_CHUNKS)))  # slop tile
    return wins


WINDOWS = _tile_windows()

# flat static slot list: (tile, kind, chunk); kind: 0=U, 1=V, 2=E
SLOTS = []
SLOT_BASE = []
for t in range(NT_ALL):
    SLOT_BASE.append(len(SLOTS))
    for c in WINDOWS[t]:
        SLOTS.append((t, 0, c))
    for j in range(V_CHUNKS):
        SLOTS.append((t, 1, j))
    for j in range(E_CHUNKS):
        SLOTS.append((t, 2, j))
SLOT_BASE.append(len(SLOTS))
N_SLOTS = len(SLOTS)

_COMPILED = None


def _build(phases="pd"):
    import concourse.mybir as mybir
    import concourse.tile as tile
    from concourse import bacc
    from concourse.bass import ts

    dt = mybir.dt
    nc = bacc.Bacc("TRN2", target_bir_lowering=False, debug=False,
                   num_devices=N_CORES)

    ments_d = nc.dram_tensor("ments", [MENT_PAD, HIDDEN], dt.bfloat16,
                             kind="ExternalInput").ap()
    w1_d = nc.dram_tensor("w1p", [W1_ROWS_PAD, HIDDEN], dt.bfloat16,
                          kind="ExternalInput").ap()
    w2b_d = nc.dram_tensor("w2b", [128, HC], dt.bfloat16,
                           kind="ExternalInput").ap()
    b2_d = nc.dram_tensor("b2", [1], dt.float32, kind="ExternalInput").ap()
    edt_d = nc.dram_tensor("edt", [32, ED_PAD], dt.bfloat16,
                           kind="ExternalInput").ap()
    vals_d = nc.dram_tensor("vals", [1, N_SLOTS * T], dt.bfloat16,
                            kind="ExternalInput").ap()
    iota_d = nc.dram_tensor("iota", [128, 1], dt.float32,
                            kind="ExternalInput").ap()
    out_d = nc.dram_tensor("out", [NT_ALL * T], dt.float32,
                           kind="ExternalOutput").ap()

    MAXNS = max(SLOT_BASE[t + 1] - SLOT_BASE[t] for t in range(NT_ALL))

    with tile.TileContext(nc) as tc:
        with (
            tc.tile_pool(name="const", bufs=1) as cpool,
            tc.tile_pool(name="tables", bufs=1) as tpool,
        ):
            w1_sb = cpool.tile([128, W1_CHUNKS, HIDDEN], dt.bfloat16)
            w2b = cpool.tile([128, HC], dt.bfloat16)
            b2_sb = cpool.tile([1, 1], dt.float32)
            edt_sb = cpool.tile([32, ED_PAD], dt.bfloat16)
            iota_sb = cpool.tile([128, 1], dt.float32)
            ones_sb = cpool.tile([1, 128], dt.bfloat16)

            u_sb = tpool.tile([128, M_CHUNKS * HIDDEN], dt.bfloat16)
            v_sb = tpool.tile([128, V_CHUNKS * HIDDEN], dt.bfloat16)
            e_sb = tpool.tile([128, E_CHUNKS * HIDDEN], dt.bfloat16)

            nc.sync.dma_start(b2_sb[:], b2_d[:])
            nc.sync.dma_start(w2b[:], w2b_d[:])
            nc.sync.dma_start(edt_sb[:], edt_d[:])
            nc.sync.dma_start(iota_sb[:], iota_d[:])
            nc.vector.memset(ones_sb[:], 1.0)
            nc.sync.dma_start(
                w1_sb[:], w1_d.rearrange("(c p) h -> p c h", p=128))

            with (
                tc.tile_pool(name="mentT", bufs=1) as mtpool,
                tc.tile_pool(name="psA", bufs=4, space="PSUM") as psA,
            ):
                mentT = []
                for k in range(HC):
                    mt = mtpool.tile([128, MENT_PAD], dt.bfloat16,
                                     tag=f"mt{k}", name=f"mentT{k}")
                    nc.sync.dma_start(mt[:], ments_d[:, ts(k, 128)],
                                      transpose=True)
                    mentT.append(mt)

                # ---- E' = [ed^T; 1].T @ [W1c; b1]  (26 contraction rows) ----
                for m in range(E_CHUNKS if "p" in phases else 0):
                    p5 = psA.tile([128, 512], dt.float32, tag="p5")
                    p2 = psA.tile([128, 256], dt.float32, tag="p2")
                    lhs = edt_sb[:META + 1, ts(m, 128)]
                    nc.tensor.matmul(p5[:], lhs, w1_sb[:META + 1, 12, :512],
                                     start=True, stop=True)
                    nc.tensor.matmul(p2[:], lhs, w1_sb[:META + 1, 12, 512:],
                                     start=True, stop=True)
                    nc.vector.tensor_copy(e_sb[:, m * HIDDEN:m * HIDDEN + 512],
                                          p5[:])
                    nc.vector.tensor_copy(
                        e_sb[:, m * HIDDEN + 512:(m + 1) * HIDDEN], p2[:])

                # ---- U (16 chunks) and V (first 4 chunks) projections ----
                for r in range(M_CHUNKS if "p" in phases else 0):
                    u5 = psA.tile([128, 512], dt.float32, tag="p5")
                    u2 = psA.tile([128, 256], dt.float32, tag="p2")
                    do_v = r < V_CHUNKS
                    if do_v:
                        v5 = psA.tile([128, 512], dt.float32, tag="p5")
                        v2 = psA.tile([128, 256], dt.float32, tag="p2")
                    for k in range(HC):
                        lhs = mentT[k][:, ts(r, 128)]
                        st0, sp1 = (k == 0), (k == HC - 1)
                        nc.tensor.matmul(u5[:], lhs, w1_sb[:, k, :512],
                                         start=st0, stop=sp1)
                        nc.tensor.matmul(u2[:], lhs, w1_sb[:, k, 512:],
                                         start=st0, stop=sp1)
                        if do_v:
                            nc.tensor.matmul(v5[:], lhs, w1_sb[:, 6 + k, :512],
                                             start=st0, stop=sp1)
                            nc.tensor.matmul(v2[:], lhs, w1_sb[:, 6 + k, 512:],
                                             start=st0, stop=sp1)
                    ro = r * HIDDEN
                    nc.vector.tensor_copy(u_sb[:, ro:ro + 512], u5[:])
                    nc.vector.tensor_copy(u_sb[:, ro + 512:ro + HIDDEN], u2[:])
                    if do_v:
                        nc.scalar.copy(v_sb[:, ro:ro + 512], v5[:])
                        nc.scalar.copy(v_sb[:, ro + 512:ro + HIDDEN], v2[:])

            # ---- pair tiles: build one-hots + expand + relu + dot ----
            with (
                tc.tile_pool(name="oh", bufs=2) as ohpool,
                tc.tile_pool(name="vt", bufs=2) as vtpool,
                tc.tile_pool(name="h", bufs=6) as hpool,
                tc.tile_pool(name="o", bufs=2) as opool,
                tc.tile_pool(name="psD", bufs=4, space="PSUM") as psD,
                tc.tile_pool(name="psB", bufs=2, space="PSUM") as psB,
                tc.tile_pool(name="psL", bufs=2, space="PSUM") as psL,
            ):
                relu = mybir.ActivationFunctionType.Relu
                ident = mybir.ActivationFunctionType.Identity
                eq = mybir.AluOpType.is_equal
                if "d" not in phases:
                    for t in range(NT_ALL):
                        lt = opool.tile([1, T], dt.float32, tag="lt")
                        nc.vector.memset(lt[:], 0.0)
                        nc.sync.dma_start(out_d[ts(t, T)], lt[:])
                for t in range(NT_ALL if "d" in phases else 0):
                    base = SLOT_BASE[t]
                    ns = SLOT_BASE[t + 1] - base
                    vt = vtpool.tile([1, MAXNS, T], dt.bfloat16, tag="vt")
                    nc.sync.dma_start(
                        vt[:1, :ns, :],
                        vals_d[:, base * T:(base + ns) * T]
                        .rearrange("o (s c) -> o s c", c=T))
                    oh_t = ohpool.tile([128, MAXNS, T], dt.bfloat16, tag="oh")
                    for s in range(ns):
                        pb = psB.tile([128, T], dt.float32, tag="pb")
                        nc.tensor.matmul(pb[:], ones_sb[:], vt[:1, s, :],
                                         start=True, stop=True)
                        nc.vector.tensor_scalar(oh_t[:, s, :], pb[:],
                                                iota_sb[:], None, eq)
                    pl = psL.tile([1, T], dt.float32, tag="pl")
                    for hc in range(HC):
                        ph = psD.tile([128, T], dt.float32, tag="ph")
                        for s in range(ns):
                            _, kind, c = SLOTS[base + s]
                            tab = (u_sb, v_sb, e_sb)[kind]
                            lhs = tab[:, c * HIDDEN + hc * 128:
                                      c * HIDDEN + (hc + 1) * 128]
                            nc.tensor.matmul(ph[:], lhs, oh_t[:, s, :],
                                             start=(s == 0), stop=(s == ns - 1))
                        h_sb = hpool.tile([128, T], dt.bfloat16, tag="h")
                        nc.scalar.activation(h_sb[:], ph[:], relu)
                        nc.tensor.matmul(pl[:], w2b[:, hc:hc + 1], h_sb[:],
                                         start=(hc == 0), stop=(hc == HC - 1))
                    lt = opool.tile([1, T], dt.float32, tag="lt")
                    nc.scalar.activation(lt[:], pl[:], ident,
                                         bias=b2_sb[:1, :1])
                    nc.sync.dma_start(out_d[ts(t, T)], lt[:])

    nc.compile()
    return nc


def _get_compiled():
    global _COMPILED
    if _COMPILED is None:
        _COMPILED = _build()
    return _COMPILED


def _assign(core_pairs_a):
    """Place pairs into quota slots by a-chunk; overflow -> slop tile."""
    n = len(core_pairs_a)
    pos = np.full(n, -1, np.int64)
    ah = core_pairs_a // 128
    slop_next = NT_Q * T
    for c in range(M_CHUNKS):
        idx = np.nonzero(ah == c)[0]
        k = min(len(idx), QUOTAS[c])
        pos[idx[:k]] = QCUM[c] + np.arange(k)
        for i in idx[k:]:
            assert slop_next < NT_Q * T + SLOP_CAP, "slop overflow"
            pos[i] = slop_next
            slop_next += 1
    return pos


_SLOT_OF = {(t, kind, c): s for s, (t, kind, c) in enumerate(SLOTS)}


def make_in_maps(mention_reprs, coref_mention_pairs, coref_eds, ed_table,
                 W1, b1, W2, b2):
    import ml_dtypes

    bf16 = ml_dtypes.bfloat16
    mention_reprs = np.asarray(mention_reprs, dtype=np.float32)
    pairs = np.asarray(coref_mention_pairs).astype(np.int64)
    eds = np.asarray(coref_eds).astype(np.int64)
    W1 = np.asarray(W1, dtype=np.float32)
    W2 = np.asarray(W2, dtype=np.float32)
    b1 = np.asarray(b1, dtype=np.float32).reshape(HIDDEN)
    b2 = np.asarray(b2, dtype=np.float32)
    ed_table = np.asarray(ed_table, dtype=np.float32)

    w1p = np.zeros((W1_ROWS_PAD, HIDDEN), np.float32)
    w1p[:W1.shape[0]] = W1
    w1p[W1.shape[0]] = b1                      # b1 folded (row 1561)
    edt = np.zeros((32, ED_PAD), np.float32)
    edt[:META, :ed_table.shape[0]] = ed_table.T
    edt[META, :] = 1.0                         # ones row -> picks up b1
    w2b = np.ascontiguousarray(W2.reshape(HC, 128).T)  # [p, c] = W2[c*128+p]
    iota = np.arange(128, dtype=np.float32).reshape(128, 1)

    shared = {
        "w1p": w1p.astype(bf16),
        "w2b": w2b.astype(bf16),
        "b2": b2.reshape(1),
        "edt": edt.astype(bf16),
        "iota": iota,
    }

    in_maps = []
    placements = []
    for core in range(N_CORES):
        b = core // SLICES
        q = core % SLICES
        bucket = np.arange(512 * q, min(512 * (q + 1), N_MENT))
        rest = np.concatenate([np.arange(0, 512 * q),
                               np.arange(min(512 * (q + 1), N_MENT), N_MENT)])
        perm = np.concatenate([bucket, rest])
        inv_perm = np.empty(N_MENT, np.int64)
        inv_perm[perm] = np.arange(N_MENT)

        ments = np.zeros((MENT_PAD, HIDDEN), np.float32)
        ments[:N_MENT] = mention_reprs[b][perm]

        bsel = (pairs[b, :, 1] >= 512 * q) & (pairs[b, :, 1] < 512 * (q + 1))
        psel = np.nonzero(bsel)[0]
        a_new = inv_perm[pairs[b, psel, 0]]
        b_loc = inv_perm[pairs[b, psel, 1]]
        e_val = eds[b, psel]

        pos = _assign(a_new)
        tile_i = pos // T
        col_i = pos % T

        vals = np.full((N_SLOTS, T), NOMATCH, np.float32)
        su = np.array([_SLOT_OF[(t, 0, c)]
                       for t, c in zip(tile_i, a_new // 128)])
        sv = np.array([_SLOT_OF[(t, 1, c)]
                       for t, c in zip(tile_i, b_loc // 128)])
        se = np.array([_SLOT_OF[(t, 2, c)]
                       for t, c in zip(tile_i, e_val // 128)])
        vals[su, col_i] = a_new % 128
        vals[sv, col_i] = b_loc % 128
        vals[se, col_i] = e_val % 128

        placements.append((psel, b, pos))
        in_maps.append({"ments": ments.astype(bf16),
                        "vals": vals.reshape(1, -1).astype(bf16),
                        **shared})
    make_in_maps.placements = placements
    return in_maps


def unshard(results, placements):
    out = np.zeros((B, N_PAIRS), np.float32)
    for core in range(N_CORES):
        psel, b, pos = placements[core]
        vals = results[core]["out"]
        out[b, psel] = vals[pos]
    return out


def kernel(**inputs):
    from concourse.bass_utils import run_bass_kernel_spmd

    nc = _get_compiled()
    in_maps = make_in_maps(**inputs)
    placements = make_in_maps.placements
    res = run_bass_kernel_spmd(nc, in_maps, list(range(N_CORES)))
    return unshard(res.results, placements)
